# revision 47
# baseline (speedup 1.0000x reference)
"""Trainium2 Bass kernel for the exp-kernel multivariate Hawkes process
log-likelihood (B=8, N=2048, D=10).

Strategy (v13)
--------------
Data-parallel over batch: core b computes batch row b fully on-chip and
returns per-event partials [128,17]; the host reduces them and adds the
-T*sum(mu) constant (unshard step).

Host ships per-event GATHERED tables (index lookups + O(N*D) products,
no transcendental math on N):
  bcol2[r,c]   = b[r, e_j]  (r-major)             (au -> u = exp)
  vabarg[m,c]  = -b[e_j,m]*trel_j + ln(ab[e_j,m]) (vab = exp, one Act op)
  negarg[c,d]  = b[d,e_j]*(t_j-T) + ln(a[d,e_j])  (neg part = exp+accum)
so the DVE keeps ONLY the chain au -> W -> mask -> r-tree -> t2/lamr/lam
plus dw+scan; the Act engine handles every other exponential.  The
negative log-likelihood part is ONE activation with accum_out (a [P,1]
running sum), subtracted from the shipped asumtot on the DVE in a [P,1]
op.  GPSIMD only builds triu during the DMA wait (it contends with the
DVE for the SBUF port, so it gets no steady-state work).

The whole pair-grid pipeline lives in (m, r, c) layout with the CHUNK
axis innermost: both W-build operands are then inner-step-1 bf16, which
unlocks the DVE's 2x packed mode (an (c,m,r) layout leaves a broadcast
step-0 operand and runs 1x).  The in-chunk prefix is grouped by
m-slices (3/2/3/2 per PSUM bank) so each matmul's moving operand is a
CONTIGUOUS W block (strided column walks measurably slow the PE).
Group 0 of the masked tail reads straight from PSUM (1x but no copy
dependency - it fills the DVE bubble while Act copies groups 1-3); the
r-contraction is a pairwise ADD-TREE that stays in 2x mode (16-element
runs), and the final m-contraction writes t2 strided back to (c, m) so
a plain innermost tensor_reduce finishes the job.

Inter-chunk state S lives in [10_r, (10_m, 16_k)] layout: chunk sums
from 16 tiny u^T@ohT2 matmuls (strided PSUM writes), the affine
recurrence S_{k+1}=d_k(S_k+w_k) is ONE tensor_tensor_scan with a k=0
decay-reset column, and S is gathered per event with 15 onehotT
matmuls straight into (m, c) layout via strided PSUM out APs.

All input DMAs are serialized on the sync queue in dependency order:
the DMA engines round-robin across queues, so any concurrent transfer
would steal bandwidth from the critical first table (and add run-to-run
jitter).  A manually emitted InstLoadActFuncSet(id=6) loads the
combined exp+ln table once.
"""
import numpy as np
from contextlib import ExitStack

import ml_dtypes
import concourse.bass as bass
import concourse.mybir as mybir
import concourse.tile as tile
from concourse import bacc
from concourse.bass_utils import run_bass_kernel_spmd
from concourse.masks import make_upper_triangular

f32 = mybir.dt.float32
bf16 = mybir.dt.bfloat16
AL = mybir.AluOpType
AF = mybir.ActivationFunctionType
AX = mybir.AxisListType

P = 128          # partitions == chunk size
KC = 16          # number of chunks
D = 10           # event types
N = P * KC       # 2048 events per batch row
B = 8            # batch == cores
NG = 4           # chunk groups (4 chunks per PSUM bank)

# packed DRAM inputs: name -> (shape, dtype)
INPUTS = {
    "hot_f32": ((P, 176), f32),    # trel(16) bcol(160)
    "hot_bf": ((P, 160), bf16),    # ohT2(160)
    "rest_f32": ((P, 340), f32),   # vabarg(160) negarg(160) musub_ev(16)
                                   # asumtot(1) pad(3)
    "oht": ((D, N + 320), bf16),   # onehotT [D,N] | decay args true |
                                   # decay args k0-killed
}


def _body(ctx: ExitStack, tc, ins, out_ap):
    nc = tc.nc
    cpool = ctx.enter_context(tc.tile_pool(name="cpool", bufs=1))
    wpool = ctx.enter_context(tc.tile_pool(name="wpool", bufs=1))
    pp = ctx.enter_context(tc.tile_pool(name="pp", bufs=1, space="PSUM"))

    # one combined exp+ln activation table load (id 6 =
    # natural_log_exp_and_others) emitted first on the Act queue
    nc.scalar.add_instruction(mybir.InstLoadActFuncSet(
        name=nc.get_next_instruction_name(), act_func_set_id=6,
        ins=[], outs=[]))

    # ---- input DMAs: ALL serialized on the sync queue in dependency
    # order.  The DMA engines round-robin across queues, so any
    # concurrent transfer steals bandwidth from the critical hot_f32;
    # one queue removes both the contention and its run-to-run jitter.
    # Later tables (oht, rest_f32) are only needed mid-kernel. ----
    hot_f32 = cpool.tile([P, 176], f32, tag="hot_f32")
    nc.sync.dma_start(out=hot_f32[:], in_=ins["hot_f32"])
    hot_bf = cpool.tile([P, 160], bf16, tag="hot_bf")
    nc.sync.dma_start(out=hot_bf[:], in_=ins["hot_bf"])
    oht = cpool.tile([D, N + 320], bf16, tag="oht")
    nc.sync.dma_start(out=oht[:], in_=ins["oht"])
    rest_f32 = cpool.tile([P, 340], f32, tag="rest_f32")
    nc.sync.dma_start(out=rest_f32[:], in_=ins["rest_f32"])

    trel = hot_f32[:, 0:16]
    # bcol2[p, r, c] = b[r, e_{p,c}] (r-major so the W build is 2x)
    bcol2 = hot_f32[:, 16:176].rearrange("p (r c) -> p r c", r=D)
    ohT2 = hot_bf[:, 0:160].rearrange("p (m c) -> p m c", m=D)
    vabarg = rest_f32[:, 0:160]
    negarg = rest_f32[:, 160:320]
    musub_ev = rest_f32[:, 320:336]
    asumtot = rest_f32[:, 336:337]

    # triu built on-device while the DMAs are in flight (gpsimd is idle)
    triu = wpool.tile([P, P], bf16, tag="triu")
    make_upper_triangular(nc, triu[:], val=1.0, diag=True)

    # ---- critical chain: au -> eu (c-halves) -> W (c-halves) -> prefix.
    # Everything in (m, r, c)/(r, c) layout with c innermost so BOTH W
    # operands are inner-step-1 bf16 and the TT runs in 2x DVE mode ----
    au = wpool.tile([P, D, KC], f32, tag="au")
    nc.vector.tensor_tensor(
        out=au[:], in0=bcol2,
        in1=trel.unsqueeze(1).broadcast_to([P, D, KC]), op=AL.mult)
    u = wpool.tile([P, D, KC], bf16, tag="u")
    nc.scalar.activation(u[:], au[:], AF.Exp)
    # decays + vab right here: they fill the Act window before Pg0 lands
    decays = wpool.tile([D, 320], f32, tag="decays")
    nc.scalar.activation(decays[:], oht[:, N:N + 320], AF.Exp, scale=-1.0)
    vab2 = wpool.tile([P, D, KC], bf16, tag="vab2")
    nc.scalar.activation(vab2[:].rearrange("p m c -> p (m c)"),
                         vabarg, AF.Exp)
    # W[j,(m,r,c)] = [e_j == m] * u[j,r,c] in m-halves (2x mode)
    W = wpool.tile([P, D, D, KC], bf16, tag="W")
    for h in range(2):
        ms = slice(5 * h, 5 * (h + 1))
        nc.vector.tensor_tensor(
            out=W[:, ms],
            in0=ohT2[:, ms].unsqueeze(2).broadcast_to([P, 5, D, KC]),
            in1=u[:].unsqueeze(1).broadcast_to([P, 5, D, KC]),
            op=AL.mult)

    # chunk sums straight into scan layout: wsq[r, m, k]
    wsq = pp.tile([D, D, KC], f32, tag="wsq", name="wsq")
    for k in range(KC):
        nc.tensor.matmul(wsq[:, :, k], u[:, :, k], ohT2[:, :, k],
                         start=True, stop=True)

    # ---- in-chunk inclusive prefix (PE), grouped by m-slices (3/2/3/2
    # per PSUM bank) so the moving operand is a CONTIGUOUS W block and
    # the PSUM result keeps the (m, r, c) layout ----
    MG = [(0, 3), (3, 5), (5, 8), (8, 10)]
    Pg = [pp.tile([P, m1 - m0, D, KC], f32, tag=f"Pg{g}", name=f"Pg{g}")
          for g, (m0, m1) in enumerate(MG)]
    for g, (m0, m1) in enumerate(MG):
        nc.tensor.matmul(Pg[g][:], triu[:], W[:, m0:m1],
                         start=True, stop=True)

    # ---- Act: per-group PSUM->SBUF copies; neg exp pushed late via a
    # scheduler wait hint so it cannot delay the copies ----
    PgSB = wpool.tile([P, D, D, KC], bf16, tag="PgSB")
    negex = wpool.tile([P, KC, D], bf16, tag="negex")
    negsum = wpool.tile([P, 1], f32, tag="negsum")
    # group 0 is consumed straight from PSUM by the DVE (it fills the
    # DVE's wait-for-copies hole); only groups 1-3 need Act copies
    for g, (m0, m1) in enumerate(MG):
        if g == 0:
            continue
        nc.scalar.copy(PgSB[:, m0:m1], Pg[g][:])
    # neg part after the copies (its consumer is the final out DMA)
    with tc.tile_wait_until(0.008):
        nc.scalar.activation(negex[:].rearrange("p c m -> p (c m)"),
                             negarg, AF.Exp, accum_out=negsum[:])

    # ---- dw + inter-chunk affine scan (DVE; gpsimd cannot) ----
    dw = wpool.tile([D, D, KC], f32, tag="dw")
    nc.vector.tensor_tensor(
        out=dw[:], in0=decays[:, 0:160].rearrange("p (m k) -> p m k", m=D),
        in1=wsq[:], op=AL.mult)
    # S_{k+1} = d0op_k * S_k + d_k*w_k  (d0op kills state at k=0 per m)
    Sout = wpool.tile([D, D * KC], bf16, tag="Sout")
    nc.vector.tensor_tensor_scan(
        Sout[:], decays[:, 160:320],
        dw[:].rearrange("p m k -> p (m k)"), initial=0.0,
        op0=AL.mult, op1=AL.add)
    Soutv = Sout[:].rearrange("p (m k) -> p m k", m=D)

    # ---- gather inter-chunk state per event straight into (m, c)
    # layout via a strided PSUM out AP: Sgall[i, m, k] = S_k[e_i, m] ----
    Sgall = pp.tile([P, D, KC], f32, tag="Sgall", name="Sgall")
    nc.vector.memset(Sgall[:, :, 0:1], 0.0)
    for k in range(1, KC):
        nc.tensor.matmul(Sgall[:, :, k], oht[:, k * P:(k + 1) * P],
                         Soutv[:, :, k - 1], start=True, stop=True)

    # ---- tail in (m, r, c): mask by onehot_r, then contract r with a
    # pairwise ADD-TREE that stays in 2x mode (16-elem runs).  The
    # first m-group masks straight from PSUM (1x but no copy wait);
    # the rest run at 2x from the bf16 copies ----
    t1 = wpool.tile([P, D, D, KC], bf16, tag="t1")
    m0, m1 = MG[0]
    nc.vector.tensor_tensor(
        out=t1[:, m0:m1], in0=Pg[0][:],
        in1=ohT2[:].unsqueeze(1).broadcast_to([P, m1 - m0, D, KC]),
        op=AL.mult)
    for g, (m0, m1) in enumerate(MG):
        if g == 0:
            continue
        nc.vector.tensor_tensor(
            out=t1[:, m0:m1], in0=PgSB[:, m0:m1],
            in1=ohT2[:].unsqueeze(1).broadcast_to([P, m1 - m0, D, KC]),
            op=AL.mult)
    # r-tree: 10 -> 5 -> (2,2,hold 1) -> 1
    A1 = wpool.tile([P, D, 5, KC], bf16, tag="A1")
    nc.vector.tensor_tensor(out=A1[:], in0=t1[:, :, 0:5],
                            in1=t1[:, :, 5:10], op=AL.add)
    A2 = wpool.tile([P, D, 2, KC], bf16, tag="A2")
    nc.vector.tensor_tensor(out=A2[:], in0=A1[:, :, 0:2],
                            in1=A1[:, :, 2:4], op=AL.add)
    A3 = wpool.tile([P, D, KC], bf16, tag="A3")
    nc.vector.tensor_tensor(out=A3[:], in0=A2[:, :, 0],
                            in1=A2[:, :, 1], op=AL.add)
    Q2 = wpool.tile([P, D, KC], bf16, tag="Q2")
    nc.vector.tensor_tensor(out=Q2[:], in0=A3[:], in1=A1[:, :, 4],
                            op=AL.add)
    # add the gathered inter-chunk state, multiply by vab
    qsum = wpool.tile([P, D, KC], f32, tag="qsum")
    nc.vector.tensor_tensor(out=qsum[:], in0=Q2[:], in1=Sgall[:],
                            op=AL.add)
    # t2 written STRIDED into (c, m) layout (f32 TT is 1x regardless, a
    # strided DVE write costs nothing extra) so the m-contraction is a
    # plain innermost tensor_reduce
    t2 = wpool.tile([P, KC, D], f32, tag="t2")
    nc.vector.tensor_tensor(out=t2[:].rearrange("p c m -> p m c"),
                            in0=qsum[:], in1=vab2[:], op=AL.mult)
    lamr = wpool.tile([P, KC], f32, tag="lamr")
    nc.vector.tensor_reduce(out=lamr[:], in_=t2[:], axis=AX.X, op=AL.add)
    lam = wpool.tile([P, KC], f32, tag="lam")
    nc.vector.tensor_tensor(out=lam[:], in0=lamr[:], in1=musub_ev,
                            op=AL.add)

    lamns = wpool.tile([P, 17], f32, tag="lamns")
    # negative part: sum_{c,m} exp(negarg) - asumtot   ([P,1])
    nc.vector.tensor_tensor(out=lamns[:, 16:17], in0=negsum[:],
                            in1=asumtot, op=AL.subtract)
    nc.scalar.activation(lamns[:, 0:16], lam[:], AF.Ln)
    nc.scalar.dma_start(out=out_ap, in_=lamns[:])


_CACHE = {}


def _build(Tval: float = 0.0):
    key = 0
    if key in _CACHE:
        return _CACHE[key]
    nc = bacc.Bacc("TRN2", target_bir_lowering=False, debug=False)
    ins = {}
    for name, (shape, dt) in INPUTS.items():
        ins[name] = nc.dram_tensor(name, list(shape), dt,
                                   kind="ExternalInput").ap()
    out_ap = nc.dram_tensor("out", [P, 17], f32,
                            kind="ExternalOutput").ap()
    with tile.TileContext(nc) as tc:
        with ExitStack() as ctx:
            _body(ctx, tc, ins, out_ap)
    nc.compile()
    _CACHE[key] = (nc, ins, out_ap)
    return _CACHE[key]


def make_in_maps(time_points, event_types, mu_raw, log_alpha, log_beta, T):
    Tval = float(np.asarray(T))
    tp = np.asarray(time_points, dtype=np.float32)          # [B, N]
    et = np.asarray(event_types).astype(np.int64)           # [B, N]

    # O(D^2) parameter transforms in float64 -> float32
    mu = np.log1p(np.exp(np.float64(mu_raw))).astype(np.float32)
    al = np.log1p(np.exp(np.float64(log_alpha))).astype(np.float32)
    be = np.log1p(np.exp(np.float64(log_beta))).astype(np.float32)
    ab = (al * be).astype(np.float32)
    musub = mu - np.diag(ab)                                # [D]
    asum = al.sum(axis=0)                                   # [D]
    beT = np.ascontiguousarray(be.T)
    lab = np.log(ab).astype(np.float32)                     # ln(alpha*beta)
    laT = np.ascontiguousarray(np.log(al).T.astype(np.float32))

    in_maps = []
    for b in range(B):
        e = et[b]                                           # [N]
        t = tp[b]
        ts = t[::P]                                         # [KC]
        dtb = np.zeros(KC, dtype=np.float32)
        dtb[:-1] = ts[1:] - ts[:-1]

        # [p, c] views (event j = c*128 + p)
        t2 = t.reshape(KC, P).T                             # [P, KC]
        e2 = e.reshape(KC, P).T                             # [P, KC]
        trel = t2 - ts[None, :]                             # [P, KC]
        tau2 = t2 - np.float32(Tval)                        # [P, KC]

        hot_f32 = np.zeros((P, 176), dtype=np.float32)
        hot_f32[:, 0:16] = trel
        # bcol2[p, r, c] = b[r, e2[p,c]] (r-major layout)
        hot_f32[:, 16:176] = beT[e2].transpose(0, 2, 1).reshape(P, KC * D)

        oh = (e2[:, :, None] == np.arange(D)[None, None, :])
        # ohT2[p, m, c]: m-major onehot for the (m,r,c)-layout pipeline
        hot_bf = np.zeros((P, 160), dtype=ml_dtypes.bfloat16)
        hot_bf[:, 0:160] = oh.transpose(0, 2, 1).reshape(P, KC * D)

        # fused exp args (products of gathered tables: O(N*D) muls/adds)
        # vabarg in (m, c) layout to match the (m, r, c) device pipeline
        vabarg = (-be[e2] * trel[:, :, None] + lab[e2]
                  ).transpose(0, 2, 1).reshape(P, KC * D)
        negarg = (beT[e2] * tau2[:, :, None] + laT[e2]).reshape(P, KC * D)
        rest_f32 = np.zeros((P, 340), dtype=np.float32)
        rest_f32[:, 0:160] = vabarg
        rest_f32[:, 160:320] = negarg
        rest_f32[:, 320:336] = musub[e2]
        rest_f32[:, 336] = asum[e2].sum(axis=1)             # asumtot

        oht = np.zeros((D, N + 320), dtype=ml_dtypes.bfloat16)
        oht[:, 0:N] = (e[None, :] == np.arange(D)[:, None])
        bdtb = be[:, :, None] * dtb[None, None, :]          # [D, D, KC]
        oht[:, N:N + 160] = bdtb.reshape(D, D * KC)
        bk0 = bdtb.copy()
        bk0[:, :, 0] = 40.0                                 # exp(-40) ~ 0
        oht[:, N + 160:N + 320] = bk0.reshape(D, D * KC)

        in_maps.append({"hot_f32": hot_f32, "hot_bf": hot_bf,
                        "rest_f32": rest_f32, "oht": oht})
    negconst = np.float32(-Tval * mu.astype(np.float64).sum())
    return in_maps, Tval, negconst


def kernel(time_points, event_types, mu_raw, log_alpha, log_beta, T):
    in_maps, Tval, negconst = make_in_maps(
        time_points, event_types, mu_raw, log_alpha, log_beta, T)
    nc, _, _ = _build(Tval)
    res = run_bass_kernel_spmd(nc, in_maps, list(range(B))).results
    out = np.array([res[b]["out"].sum() + negconst for b in range(B)],
                   dtype=np.float32)  # loglam + neg part both summed
    return out
